# revision 1
# baseline (speedup 1.0000x reference)
"""CapsuleLayer dynamic-routing kernel for 8 Trainium2 NeuronCores.

Math (reference):
    u_hat[b,n,j,d] = sum_i W[n,j,d,i] * x[b,j,i]
    b = 0; for r in 0..2:
        c = softmax_n(b); s[b,n,d] = sum_j c*u_hat; v = squash_d(s)
        if r < 2: b += sum_d v*u_hat
    return v  [B, N, D]

Sharding: J (input capsules, 2048) split 8 ways -> Jc=256 per core.
Softmax over n is local; only s needs a 256 KiB AllReduce per iteration.

Per-core dataflow, one sweep over W per routing iteration (u_hat is
recomputed from SBUF-streamed W each iteration; never materialized):
  - j processed in groups of 4: 4 PE sub-matmuls via column tiling
    (tile_position=(0,32r)) produce u_hat group tile
    [128=(4j x 32b), (n,d)] in PSUM.
  - r0: softmax(0) is uniform, so u_hat is accumulated over all j
    directly in PSUM; s0 = (1/N) * strip-sum. No vector work at all.
  - r>=1: logits[p=(j,b), (g,n)] += sum_d v_{r-1}*u_hat  (DVE mult +
    segmented reduce over d); softmax over n is local to each
    (partition, group) -> c; tmp2 = c (x) u_hat on GpSimd.
  - s accumulated over j by a PE matmul with a stacked-identity lhsT
    (sums the 4 j-strips per b), accumulating across groups in PSUM.
    s-matmuls are emitted one group late so they don't block the next
    group's u_hat matmuls in the in-order PE queue.
  - AllReduce s across cores, squash redundantly on every core.
"""

import functools
import numpy as np

B, J, I = 32, 2048, 16
N, D = 64, 32
NCORES = 8
JC = J // NCORES          # 256 j per core
GRP = 4                   # j's per group (PE column strips)
NG = JC // GRP            # 64 groups
ND = N * D                # 2048
HALF = ND // 2            # 1024 free-dim half (PSUM sizing)
NH = N // 2               # 32 n per half
ROUTINGS = 3
EPS = 1e-7


@functools.lru_cache(maxsize=1)
def _build():
    import concourse.bass as bass
    import concourse.mybir as mybir
    import concourse.bacc as bacc
    import concourse.tile as tile

    f32 = mybir.dt.float32
    bf16 = mybir.dt.bfloat16
    MUL = mybir.AluOpType.mult
    ADD = mybir.AluOpType.add
    AX = mybir.AxisListType.X
    AF = mybir.ActivationFunctionType

    nc = bacc.Bacc("TRN2", target_bir_lowering=False, debug=False,
                   num_devices=NCORES)

    xt_d = nc.dram_tensor("xt", [I, JC * B], bf16, kind="ExternalInput")
    wt_d = nc.dram_tensor("wt", [I, JC, ND], bf16, kind="ExternalInput")
    ones_d = nc.dram_tensor("ones4", [GRP * B, B], bf16, kind="ExternalInput")
    v_d = nc.dram_tensor("v", [B, ND], f32, kind="ExternalOutput")

    with tile.TileContext(nc) as tc:
        with (
            tc.tile_pool(name="persist", bufs=1) as pp,
            tc.tile_pool(name="wstream", bufs=4) as wp,
            tc.tile_pool(name="work", bufs=4) as wk,
            tc.tile_pool(name="small", bufs=6) as sm,
            tc.tile_pool(name="ups", bufs=3, space="PSUM") as ups_pool,
            tc.tile_pool(name="sps", bufs=1, space="PSUM") as sps_pool,
            tc.tile_pool(name="dram", bufs=1, space="DRAM") as dr,
        ):
            xt = pp.tile([I, JC * B], bf16)
            nc.sync.dma_start(xt[:], xt_d[:])
            ones4 = pp.tile([GRP * B, B], bf16)
            nc.sync.dma_start(ones4[:], ones_d[:])

            logits = pp.tile([128, NG, N], bf16)
            v_rep = pp.tile([128, N, D], bf16)
            v_small = pp.tile([B, ND], bf16)
            s_sb = pp.tile([128, 512], f32)
            v_sb = pp.tile([B, ND], f32)

            cc_in = dr.tile([128, 512], f32)
            cc_out = dr.tile([128, 512], f32)

            def u_mms(u_ps, w_t, g, h, start, stop):
                """16 col-tiled matmuls for one (group, half); rr-outer so
                consecutive chunk matmuls share the stationary lhsT."""
                for rr in range(GRP):
                    j = g * GRP + rr
                    for cch in range(2):
                        nc.tensor.matmul(
                            u_ps[32 * rr:32 * rr + 32,
                                 cch * 512:(cch + 1) * 512],
                            xt[:, j * B:(j + 1) * B],
                            w_t[:, rr, h * HALF + cch * 512:
                                h * HALF + (cch + 1) * 512],
                            start=start, stop=stop,
                            tile_position=(0, 32 * rr),
                            skip_group_check=True,
                        )

            for r in range(ROUTINGS):
                s_ps = sps_pool.tile([128, 512], f32)

                if r == 0:
                    # -- r0: c is uniform; accumulate u_hat over j in PSUM --
                    acc = [ups_pool.tile([128, HALF], f32, name=f"acc{_h}", tag="u_ps") for _h in range(2)]
                    for g in range(NG):
                        w_t = wp.tile([I, GRP, ND], bf16)
                        nc.sync.dma_start(
                            w_t[:], wt_d[:, g * GRP:(g + 1) * GRP, :])
                        for h in range(2):
                            u_mms(acc[h], w_t, g, h,
                                  start=(g == 0), stop=(g == NG - 1))
                    # evac to bf16 SBUF, then strip-sum via ones4 matmul
                    for h in range(2):
                        a_sb = wk.tile([128, HALF], bf16)
                        nc.scalar.activation(a_sb[:], acc[h][:], AF.Copy)
                        for cch in range(2):
                            q = 2 * h + cch
                            nc.tensor.matmul(
                                s_ps[32 * q:32 * q + 32, :],
                                ones4[:],
                                a_sb[:, cch * 512:(cch + 1) * 512],
                                start=True, stop=True,
                                tile_position=(0, 32 * q),
                                skip_group_check=True,
                            )
                else:
                    # -- r>=1: fused logits update + local softmax + s --
                    pending_smm = []
                    for g in range(NG):
                        w_t = wp.tile([I, GRP, ND], bf16)
                        nc.sync.dma_start(
                            w_t[:], wt_d[:, g * GRP:(g + 1) * GRP, :])

                        c_t = sm.tile([128, N], bf16)
                        zrec = sm.tile([128, 1], f32)

                        u_sb_halves = []
                        for h in range(2):
                            u_ps = ups_pool.tile([128, HALF], f32)
                            u_mms(u_ps, w_t, g, h, start=True, stop=True)
                            # emit previous group's s-matmuls behind this
                            # group's u-matmuls in the PE stream
                            if pending_smm:
                                pending_smm.pop(0)()

                            u_sb = wk.tile([128, NH, D], bf16)
                            nc.scalar.activation(u_sb[:], u_ps[:], AF.Copy)
                            tl = wk.tile([128, NH, D], bf16)
                            nc.vector.tensor_tensor(
                                tl[:], u_sb[:],
                                v_rep[:, h * NH:(h + 1) * NH, :], op=MUL)
                            with nc.allow_low_precision("bf16 routing logits"):
                                if r == 1:
                                    nc.vector.tensor_reduce(
                                        logits[:, g, h * NH:(h + 1) * NH],
                                        tl[:], axis=AX, op=ADD)
                                else:
                                    dtmp = sm.tile([128, NH], bf16)
                                    nc.vector.tensor_reduce(
                                        dtmp[:], tl[:], axis=AX, op=ADD)
                                    nc.vector.tensor_add(
                                        logits[:, g, h * NH:(h + 1) * NH],
                                        logits[:, g, h * NH:(h + 1) * NH],
                                        dtmp[:])
                            u_sb_halves.append(u_sb)

                        # local softmax over n for this group's 4 j's
                        e_t = sm.tile([128, N], f32)
                        nc.scalar.activation(e_t[:], logits[:, g, :], AF.Exp)
                        zsum = sm.tile([128, 1], f32)
                        nc.vector.tensor_reduce(zsum[:], e_t[:], axis=AX, op=ADD)
                        nc.vector.reciprocal(zrec[:], zsum[:])
                        nc.vector.tensor_scalar_mul(c_t[:], e_t[:], zrec[:])

                        # tmp2 = c (x) u_hat on GpSimd (c broadcast over d)
                        t2s = []
                        for h in range(2):
                            t2 = wk.tile([128, NH, D], bf16, name="t2", tag="t2")
                            eng = nc.vector if h == 0 else nc.gpsimd
                            eng.tensor_tensor(
                                t2[:], u_sb_halves[h][:],
                                c_t[:, h * NH:(h + 1) * NH, None]
                                .broadcast_to([128, NH, D]),
                                op=MUL)
                            t2s.append(t2)

                        def make_smm(t2s=t2s, g=g):
                            def emit():
                                for h in range(2):
                                    t2f = t2s[h][:].rearrange("p a b -> p (a b)")
                                    for cch in range(2):
                                        q = 2 * h + cch
                                        nc.tensor.matmul(
                                            s_ps[32 * q:32 * q + 32, :],
                                            ones4[:],
                                            t2f[:, cch * 512:(cch + 1) * 512],
                                            start=(g == 0), stop=(g == NG - 1),
                                            tile_position=(0, 32 * q),
                                            skip_group_check=True,
                                        )
                            return emit
                        pending_smm.append(make_smm())
                    while pending_smm:
                        pending_smm.pop(0)()

                # ---- end of sweep: AllReduce s, squash, update v ----
                # everything below stays in the (quarter, b)-strip layout:
                # partition 32q+b holds n in [16q,16q+16), all of d.
                s_evac = sm.tile([128, 512], f32)
                if r == 0:
                    nc.vector.tensor_scalar_mul(s_evac[:], s_ps[:], 1.0 / N)
                else:
                    nc.vector.tensor_copy(s_evac[:], s_ps[:])
                nc.sync.dma_start(cc_in[:], s_evac[:])
                nc.gpsimd.collective_compute(
                    "AllReduce", ADD,
                    replica_groups=[list(range(NCORES))],
                    ins=[cc_in[:].opt()], outs=[cc_out[:].opt()],
                )
                nc.sync.dma_start(s_sb[:], cc_out[:])

                sq = sm.tile([128, 16, D], f32)
                s3 = s_sb[:].rearrange("p (n d) -> p n d", d=D)
                nc.vector.tensor_tensor(sq[:], s3, s3, op=MUL)
                ns2 = sm.tile([128, 16], f32)
                nc.vector.tensor_reduce(ns2[:], sq[:], axis=AX, op=ADD)
                onep = sm.tile([128, 16], f32)
                nc.vector.tensor_scalar_add(onep[:], ns2[:], 1.0)
                rt = sm.tile([128, 16], f32)
                eps_t = sm.tile([128, 1], f32)
                nc.vector.memset(eps_t[:], EPS)
                nc.scalar.activation(rt[:], ns2[:], AF.Sqrt, bias=eps_t[:])
                den = sm.tile([128, 16], f32)
                nc.vector.tensor_tensor(den[:], onep[:], rt[:], op=MUL)
                dinv = sm.tile([128, 16], f32)
                nc.vector.reciprocal(dinv[:], den[:])
                scl = sm.tile([128, 16], f32)
                nc.vector.tensor_tensor(scl[:], ns2[:], dinv[:], op=MUL)
                v4 = sm.tile([128, 16, D], f32)
                nc.vector.tensor_tensor(
                    v4[:], s3,
                    scl[:, :, None].broadcast_to([128, 16, D]),
                    op=MUL)

                if r < ROUTINGS - 1:
                    v4b = sm.tile([128, 512], bf16)
                    nc.vector.tensor_copy(
                        v4b[:], v4[:].rearrange("p a b -> p (a b)"))
                    for q in range(4):
                        nc.sync.dma_start(
                            v_small[:, q * 512:(q + 1) * 512],
                            v4b[32 * q:32 * q + 32, :])
                    for rr in range(GRP):
                        nc.sync.dma_start(
                            v_rep[32 * rr:32 * rr + 32, :, :],
                            v_small[:].rearrange("b (n d) -> b n d", d=D))
                else:
                    for q in range(4):
                        nc.sync.dma_start(
                            v_sb[:, q * 512:(q + 1) * 512],
                            v4[32 * q:32 * q + 32, :])

            nc.sync.dma_start(v_d[:], v_sb[:])

    nc.compile()
    return nc


def kernel(x: np.ndarray, W: np.ndarray) -> np.ndarray:
    import ml_dtypes
    from concourse.bass_utils import run_bass_kernel_spmd

    nc = _build()

    bf = ml_dtypes.bfloat16
    xt = np.ascontiguousarray(x.transpose(2, 1, 0)).astype(bf)          # [I,J,B]
    wt = np.ascontiguousarray(W.transpose(3, 1, 0, 2).reshape(I, J, ND)).astype(bf)
    ones4 = np.tile(np.eye(B, dtype=np.float32), (GRP, 1)).astype(bf)

    in_maps = []
    for k in range(NCORES):
        jlo, jhi = k * JC, (k + 1) * JC
        in_maps.append({
            "xt": np.ascontiguousarray(xt[:, jlo:jhi, :]).reshape(I, JC * B),
            "wt": np.ascontiguousarray(wt[:, jlo:jhi, :]),
            "ones4": ones4,
        })

    res = run_bass_kernel_spmd(nc, in_maps, list(range(NCORES)))
    v = np.asarray(res.results[0]["v"], dtype=np.float32)
    return v.reshape(B, N, D)


if __name__ == "__main__":
    rng = np.random.default_rng(0)
    x = rng.normal(size=(B, J, I)).astype(np.float32)
    W = rng.normal(size=(N, J, D, I)).astype(np.float32) * 0.05
    v = kernel(x, W)
    print(v.shape, v.dtype, np.abs(v).max())



# revision 15
# speedup vs baseline: 1.3059x; 1.3059x over previous
"""CapsuleLayer dynamic-routing kernel for 8 Trainium2 NeuronCores.

Math (reference):
    u_hat[b,n,j,d] = sum_i W[n,j,d,i] * x[b,j,i]
    b = 0; for r in 0..2:
        c = softmax_n(b); s[b,n,d] = sum_j c*u_hat; v = squash_d(s)
        if r < 2: b += sum_d v*u_hat
    return v  [B, N, D]

Sharding: J (2048) split 8 ways -> Jc=256 per core; softmax over n is
local, only s needs a 256 KiB AllReduce per iteration.

Layout choices (measured-rate driven):
  - W streamed as wt[(j,i), (h,d,n')]: d-major free dim, so u_hat PSUM
    tiles are [(j4,b), (h,d,n')] and the e-broadcast mult (stride-0 on
    the OUTER free dim) hits the DVE 2x mode.
  - u-matmuls: block-diagonal lhsT xblk [64=(4j x 16i), 128=(j4,b)]
    -> K=64 per streamed column (4x fewer PE columns than per-j mms).
  - logits d-reduction: pairwise TT-add fold tree (stride-1 = DVE 2x)
    instead of tensor_reduce (1x only), batched over 4 groups.
  - softmax z rides act-engine exp via accum_out; 1/z folds into the
    s-matmul lhsT (zinv4 = ones4 * zrec), so t2 = e (x) u_hat.
  - r0: dense K=128 lhsT (8j chunks) accumulating s0*N directly in
    PSUM partitions 0:32; no strip-sum matmul pass.
"""

import functools
import numpy as np

B, J, I = 32, 2048, 16
N, D = 64, 32
NCORES = 8
JC = J // NCORES          # 256 j per core
GRP = 4                   # j's per group (r1/r2)
NG = JC // GRP            # 64 groups
CH8 = JC // 8             # 32 chunks of 8 j (r0)
ND = N * D                # 2048
NH = N // 2               # 32 n per half
ROUTINGS = 3
EPS = 1e-7
QUAD = 4                  # groups per fold batch

import os
NROUT = int(os.environ.get("KROUT", str(ROUTINGS)))  # debug: fewer sweeps


@functools.lru_cache(maxsize=1)
def _build():
    import concourse.bass as bass
    import concourse.mybir as mybir
    import concourse.bacc as bacc
    import concourse.tile as tile

    f32 = mybir.dt.float32
    bf16 = mybir.dt.bfloat16
    MUL = mybir.AluOpType.mult
    ADD = mybir.AluOpType.add
    AF = mybir.ActivationFunctionType

    nc = bacc.Bacc("TRN2", target_bir_lowering=False, debug=False,
                   num_devices=NCORES)

    wt_d = nc.dram_tensor("wt", [JC * I, ND], bf16, kind="ExternalInput")
    xblk_d = nc.dram_tensor("xblk", [64, NG * 128], bf16, kind="ExternalInput")
    xbl8_d = nc.dram_tensor("xbl8", [128, CH8 * B], bf16, kind="ExternalInput")
    ones_d = nc.dram_tensor("ones4", [128, B], bf16, kind="ExternalInput")
    v_d = nc.dram_tensor("v", [B, ND], f32, kind="ExternalOutput")

    with tile.TileContext(nc) as tc:
        with (
            tc.tile_pool(name="persist", bufs=1) as pp,
            tc.tile_pool(name="wstream", bufs=4) as wp,
            tc.tile_pool(name="work", bufs=2) as wk,
            tc.tile_pool(name="t2p", bufs=2) as t2p,
            tc.tile_pool(name="small", bufs=4) as sm,
            tc.tile_pool(name="zp", bufs=2) as zp,
            tc.tile_pool(name="ups", bufs=3, space="PSUM") as ups_pool,
            tc.tile_pool(name="sps", bufs=1, space="PSUM") as sps_pool,
            tc.tile_pool(name="dram", bufs=1, space="DRAM") as dr,
        ):
            xblk = pp.tile([64, NG, 128], bf16)
            nc.sync.dma_start(xblk[:], xblk_d[:])
            xbl8 = pp.tile([128, CH8, B], bf16)
            nc.sync.dma_start(xbl8[:], xbl8_d[:])
            ones4 = pp.tile([128, B], bf16)
            nc.sync.dma_start(ones4[:], ones_d[:])

            # logits [128=(j4,b), g, h, n'] bf16 (persistent across sweeps)
            logits = pp.tile([128, NG, 2, NH], bf16)
            v_rep = pp.tile([128, 2048], bf16)    # v in (h,d,n'') layout
            v_small = pp.tile([B, 2048], bf16)
            s_sb = pp.tile([128, 512], f32)       # strips (q=(h,nh), b)
            v_sb = pp.tile([B, ND], f32)

            cc_in = dr.tile([128, 512], f32)
            cc_out = dr.tile([128, 512], f32)

            def squash_update(r, s_ps):
                """AllReduce s strips, squash, write v_rep (or final v).

                Strip layout: partition 32*q + b, q = 2*h + nh;
                free = (d32, n16) d-major; n = h*32 + nh*16 + n'.
                s_ps None => cc_in already written (r0 path).
                """
                if s_ps is not None:
                    s_evac = sm.tile([128, 512], f32, name="s_evac", tag="sev")
                    nc.vector.tensor_copy(s_evac[:], s_ps[:])
                    nc.sync.dma_start(cc_in[:], s_evac[:])
                nc.gpsimd.collective_compute(
                    "AllReduce", ADD,
                    replica_groups=[list(range(NCORES))],
                    ins=[cc_in[:].opt()], outs=[cc_out[:].opt()],
                )
                nc.sync.dma_start(s_sb[:], cc_out[:])

                s3 = s_sb[:].rearrange("p (d n) -> p d n", d=D)  # [128,32,16]
                sq = sm.tile([128, D, 16], f32, name="sq", tag="sq")
                nc.vector.tensor_tensor(sq[:], s3, s3, op=MUL)
                nc.vector.tensor_tensor(
                    sq[:, 0:16, :], sq[:, 0:16, :], sq[:, 16:32, :], op=ADD)
                nc.vector.tensor_tensor(
                    sq[:, 0:8, :], sq[:, 0:8, :], sq[:, 8:16, :], op=ADD)
                nc.vector.tensor_tensor(
                    sq[:, 0:4, :], sq[:, 0:4, :], sq[:, 4:8, :], op=ADD)
                nc.vector.tensor_tensor(
                    sq[:, 0:2, :], sq[:, 0:2, :], sq[:, 2:4, :], op=ADD)
                ns2 = sm.tile([128, 16], f32, name="ns2", tag="ns2")
                nc.vector.tensor_tensor(ns2[:], sq[:, 0, :], sq[:, 1, :], op=ADD)

                onep = sm.tile([128, 16], f32, name="onep", tag="onep")
                nc.vector.tensor_scalar_add(onep[:], ns2[:], 1.0)
                eps_t = sm.tile([128, 1], f32, name="eps", tag="eps")
                nc.vector.memset(eps_t[:], EPS)
                rt = sm.tile([128, 16], f32, name="rt", tag="rt")
                nc.scalar.activation(rt[:], ns2[:], AF.Sqrt, bias=eps_t[:])
                den = sm.tile([128, 16], f32, name="den", tag="den")
                nc.vector.tensor_tensor(den[:], onep[:], rt[:], op=MUL)
                dinv = sm.tile([128, 16], f32, name="dinv", tag="dinv")
                nc.vector.reciprocal(dinv[:], den[:])
                scl = sm.tile([128, 16], f32, name="scl", tag="scl")
                nc.vector.tensor_tensor(scl[:], ns2[:], dinv[:], op=MUL)
                v4 = sm.tile([128, D, 16], f32, name="v4", tag="v4")
                nc.vector.tensor_tensor(
                    v4[:], s3,
                    scl[:, None, :].broadcast_to([128, D, 16]),
                    op=MUL)

                if r < NROUT - 1:
                    v4b = sm.tile([128, 512], bf16, name="v4b", tag="v4b")
                    nc.vector.tensor_copy(
                        v4b[:], v4[:].rearrange("p a b -> p (a b)"))
                    # strips -> v_small[b, (h, d, nh, n')] = (h, d, n'')
                    vsm = v_small[:].rearrange(
                        "b (h d nh n) -> b h d nh n", h=2, d=D, nh=2)
                    for h in range(2):
                        for nh in range(2):
                            q = 2 * h + nh
                            nc.sync.dma_start(
                                vsm[:, h, :, nh, :],
                                v4b[32 * q:32 * q + 32, :]
                                .rearrange("b (d n) -> b d n", d=D))
                    for rr in range(GRP):
                        nc.sync.dma_start(
                            v_rep[32 * rr:32 * rr + 32, :], v_small[:])
                else:
                    # final v in standard (n, d) layout
                    v4t = sm.tile([128, 16, D], f32, name="v4t", tag="v4t")
                    nc.vector.tensor_copy(
                        v4t[:], v4[:].rearrange("p d n -> p n d"))
                    vo = v_sb[:].rearrange(
                        "b (h nh n d) -> b h nh n d", h=2, nh=2, n=16)
                    for h in range(2):
                        for nh in range(2):
                            q = 2 * h + nh
                            nc.sync.dma_start(
                                vo[:, h, nh, :, :],
                                v4t[32 * q:32 * q + 32, :, :])

            for r in range(NROUT):
                if r == 0:
                    # accumulate s0*N = sum_j u_hat in PSUM partitions 0:32
                    acc = [ups_pool.tile([128, 1024], f32,
                                         name=f"acc{h}", tag="u_ps")
                           for h in range(2)]
                    for c in range(CH8):
                        w_t = wp.tile([128, 2048], bf16, name="w8", tag="w8")
                        nc.sync.dma_start(
                            w_t[:], wt_d[128 * c:128 * (c + 1), :])
                        for h in range(2):
                            for cch in range(2):
                                lo = 1024 * h + 512 * cch
                                nc.tensor.matmul(
                                    acc[h][0:32, 512 * cch:512 * (cch + 1)],
                                    xbl8[:, c, :],
                                    w_t[:, lo:lo + 512],
                                    start=(c == 0), stop=(c == CH8 - 1),
                                )
                    a_ev = wk.tile([32, 2, 1024], f32, name="a_ev", tag="a_ev")
                    for h in range(2):
                        nc.vector.tensor_scalar_mul(
                            a_ev[:, h, :], acc[h][0:32, :], 1.0 / N)
                    aev = a_ev[:].rearrange(
                        "b h (d nh n) -> b h d nh n", d=D, nh=2)
                    cci = cc_in[:].rearrange("(q b) (d n) -> q b d n", q=4, d=D)
                    for h in range(2):
                        for nh in range(2):
                            q = 2 * h + nh
                            nc.sync.dma_start(
                                cci[q, :, :, :], aev[:, h, :, nh, :])
                    squash_update(0, None)
                else:
                    s_ps = sps_pool.tile([128, 512], f32,
                                         name=f"s{r}", tag="s_ps")
                    pending_smm = []
                    for q4 in range(NG // QUAD):
                        u_sb4 = wk.tile([128, QUAD, 2048], bf16,
                                        name="u_sb4", tag="u_sb")
                        tl4 = wk.tile([128, 2 * QUAD, D, NH], bf16,
                                      name="tl4", tag="tl4")
                        for gi in range(QUAD):
                            g = q4 * QUAD + gi
                            w_t = wp.tile([64, 2048], bf16, name="w4", tag="w")
                            nc.sync.dma_start(
                                w_t[:], wt_d[64 * g:64 * (g + 1), :])
                            for h in range(2):
                                u_ps = ups_pool.tile([128, 1024], f32,
                                                     name="u_ps", tag="u_ps")
                                for cch in range(2):
                                    lo = 1024 * h + 512 * cch
                                    nc.tensor.matmul(
                                        u_ps[:, 512 * cch:512 * (cch + 1)],
                                        xblk[:, g, :],
                                        w_t[:, lo:lo + 512],
                                        start=True, stop=True,
                                    )
                                if pending_smm:
                                    pending_smm.pop(0)()
                                nc.scalar.activation(
                                    u_sb4[:, gi, 1024 * h:1024 * (h + 1)],
                                    u_ps[:], AF.Copy)
                            with nc.allow_low_precision("bf16 logits"):
                                nc.vector.tensor_tensor(
                                    tl4[:, 2 * gi:2 * gi + 2, :, :],
                                    u_sb4[:, gi, :].rearrange(
                                        "p (h d n) -> p h d n", h=2, d=D),
                                    v_rep[:].rearrange(
                                        "p (h d n) -> p h d n", h=2, d=D),
                                    op=MUL)

                        # fold over d: [128, 8, 32, 32] -> per-(g,h) rows
                        with nc.allow_low_precision("bf16 logits"):
                            nc.vector.tensor_tensor(
                                tl4[:, :, 0:16, :], tl4[:, :, 0:16, :],
                                tl4[:, :, 16:32, :], op=ADD)
                            nc.vector.tensor_tensor(
                                tl4[:, :, 0:8, :], tl4[:, :, 0:8, :],
                                tl4[:, :, 8:16, :], op=ADD)
                            nc.vector.tensor_tensor(
                                tl4[:, :, 0:4, :], tl4[:, :, 0:4, :],
                                tl4[:, :, 4:8, :], op=ADD)
                            nc.vector.tensor_tensor(
                                tl4[:, :, 0:2, :], tl4[:, :, 0:2, :],
                                tl4[:, :, 2:4, :], op=ADD)
                            for gi in range(QUAD):
                                g = q4 * QUAD + gi
                                dst = logits[:, g, :, :]
                                a0 = tl4[:, 2 * gi:2 * gi + 2, 0, :]
                                a1 = tl4[:, 2 * gi:2 * gi + 2, 1, :]
                                if r == 1:
                                    nc.vector.tensor_tensor(dst, a0, a1, op=ADD)
                                else:
                                    nc.vector.tensor_tensor(a0, a0, a1, op=ADD)
                                    nc.vector.tensor_tensor(dst, dst, a0, op=ADD)

                        for gi in range(QUAD):
                            g = q4 * QUAD + gi
                            e_t = sm.tile([128, 2, NH], bf16,
                                          name="e_t", tag="e_t")
                            zsum = sm.tile([128, 1], f32, name="zs", tag="zs")
                            nc.scalar.activation(
                                e_t[:], logits[:, g, :, :], AF.Exp,
                                accum_out=zsum[:])
                            zrec = sm.tile([128, 1], f32, name="zr", tag="zr")
                            nc.vector.reciprocal(zrec[:], zsum[:])
                            zinv4 = zp.tile([128, B], bf16, name="zi", tag="zi")
                            nc.vector.tensor_scalar_mul(
                                zinv4[:], ones4[:], zrec[:])

                            # t2 = e (x) u_hat; d-outer bcast = DVE 2x
                            t2 = t2p.tile([128, 2, D, NH], bf16,
                                          name="t2", tag="t2")
                            u3 = u_sb4[:, gi, :].rearrange(
                                "p (h d n) -> p h d n", h=2, d=D)
                            nc.vector.tensor_tensor(
                                t2[:, 0, :, :], u3[:, 0, :, :],
                                e_t[:, 0, None, :].broadcast_to([128, D, NH]),
                                op=MUL)
                            nc.gpsimd.tensor_tensor(
                                t2[:, 1, :, :], u3[:, 1, :, :],
                                e_t[:, 1, None, :].broadcast_to([128, D, NH]),
                                op=MUL)

                            def make_smm(t2=t2, zinv4=zinv4, g=g):
                                def emit():
                                    for h in range(2):
                                        for nh in range(2):
                                            qq = 2 * h + nh
                                            nc.tensor.matmul(
                                                s_ps[32 * qq:32 * qq + 32, :],
                                                zinv4[:],
                                                t2[:, h, :, 16 * nh:16 * nh + 16],
                                                start=(g == 0),
                                                stop=(g == NG - 1),
                                                tile_position=(0, 32 * qq),
                                                skip_group_check=True,
                                            )
                                return emit
                            pending_smm.append(make_smm())
                    while pending_smm:
                        pending_smm.pop(0)()
                    squash_update(r, s_ps)

            nc.sync.dma_start(v_d[:], v_sb[:])

    nc.compile()
    return nc


def _prep_inputs(x: np.ndarray, W: np.ndarray):
    import ml_dtypes
    bf = ml_dtypes.bfloat16
    in_maps = []
    ones4 = np.tile(np.eye(B, dtype=np.float32), (GRP, 1)).astype(bf)
    for k in range(NCORES):
        jlo, jhi = k * JC, (k + 1) * JC
        Wc = W[:, jlo:jhi]                      # [N, JC, D, I]
        # wt[(j,i), (h,d,n')] = W[h*32+n', j, d, i]
        wt = np.ascontiguousarray(
            Wc.reshape(2, NH, JC, D, I).transpose(2, 4, 0, 3, 1)
        ).reshape(JC * I, ND).astype(bf)
        xc = x[:, jlo:jhi, :]                   # [B, JC, I]
        xg = np.ascontiguousarray(
            xc.transpose(1, 2, 0)).reshape(NG, GRP, I, B)  # [g, rr, i, b]
        # xblk[g][(rr,i), (rr',b)] = x[b, 4g+rr, i] * delta(rr,rr')
        xblk = np.zeros((NG, GRP * I, GRP * B), dtype=np.float32)
        for rr in range(GRP):
            xblk[:, rr * I:(rr + 1) * I, rr * B:(rr + 1) * B] = xg[:, rr]
        xblk = np.ascontiguousarray(xblk.transpose(1, 0, 2)).reshape(
            GRP * I, NG * GRP * B).astype(bf)           # [64, NG*128]
        # xbl8[(jj,i), (c,b)] = x[b, 8c+jj, i] (dense)
        x8 = np.ascontiguousarray(
            xc.transpose(1, 2, 0)).reshape(CH8, 8, I, B)   # [c, jj, i, b]
        xbl8 = np.ascontiguousarray(
            x8.transpose(1, 2, 0, 3)).reshape(128, CH8 * B).astype(bf)
        in_maps.append({
            "wt": wt,
            "xblk": xblk,
            "xbl8": xbl8,
            "ones4": ones4,
        })
    return in_maps


def kernel(x: np.ndarray, W: np.ndarray) -> np.ndarray:
    from concourse.bass_utils import run_bass_kernel_spmd

    nc = _build()
    in_maps = _prep_inputs(x, W)
    res = run_bass_kernel_spmd(nc, in_maps, list(range(NCORES)))
    v = np.asarray(res.results[0]["v"], dtype=np.float32)
    return v.reshape(B, N, D)


if __name__ == "__main__":
    rng = np.random.default_rng(0)
    x = rng.normal(size=(B, J, I)).astype(np.float32)
    W = rng.normal(size=(N, J, D, I)).astype(np.float32) * 0.05
    v = kernel(x, W)
    print(v.shape, v.dtype, np.abs(v).max())


# revision 16
# speedup vs baseline: 1.4124x; 1.0815x over previous
"""CapsuleLayer dynamic-routing kernel for 8 Trainium2 NeuronCores.

Math (reference):
    u_hat[b,n,j,d] = sum_i W[n,j,d,i] * x[b,j,i]
    b = 0; for r in 0..2:
        c = softmax_n(b); s[b,n,d] = sum_j c*u_hat; v = squash_d(s)
        if r < 2: b += sum_d v*u_hat
    return v  [B, N, D]

Sharding: J (2048) split 8 ways -> Jc=256 per core; softmax over n is
local, only s needs a 256 KiB AllReduce per iteration.

Layout choices (measured-rate driven):
  - W streamed as wt[(j,i), (h,d,n')]: d-major free dim, so u_hat PSUM
    tiles are [(j4,b), (h,d,n')] and the e-broadcast mult (stride-0 on
    the OUTER free dim) hits the DVE 2x mode.
  - u-matmuls: block-diagonal lhsT xblk [64=(4j x 16i), 128=(j4,b)]
    -> K=64 per streamed column (4x fewer PE columns than per-j mms).
  - logits d-reduction: pairwise TT-add fold tree (stride-1 = DVE 2x)
    instead of tensor_reduce (1x only), batched over 4 groups.
  - softmax z rides act-engine exp via accum_out; 1/z folds into the
    s-matmul lhsT (zinv4 = ones4 * zrec), so t2 = e (x) u_hat.
  - r0: dense K=128 lhsT (8j chunks) accumulating s0*N directly in
    PSUM partitions 0:32; no strip-sum matmul pass.
"""

import functools
import numpy as np

B, J, I = 32, 2048, 16
N, D = 64, 32
NCORES = 8
JC = J // NCORES          # 256 j per core
GRP = 4                   # j's per group (r1/r2)
NG = JC // GRP            # 64 groups
CH8 = JC // 8             # 32 chunks of 8 j (r0)
ND = N * D                # 2048
NH = N // 2               # 32 n per half
ROUTINGS = 3
EPS = 1e-7
QUAD = 4                  # groups per fold batch

import os
NROUT = int(os.environ.get("KROUT", str(ROUTINGS)))  # debug: fewer sweeps


@functools.lru_cache(maxsize=1)
def _build():
    import concourse.bass as bass
    import concourse.mybir as mybir
    import concourse.bacc as bacc
    import concourse.tile as tile

    f32 = mybir.dt.float32
    bf16 = mybir.dt.bfloat16
    MUL = mybir.AluOpType.mult
    ADD = mybir.AluOpType.add
    AF = mybir.ActivationFunctionType

    nc = bacc.Bacc("TRN2", target_bir_lowering=False, debug=False,
                   num_devices=NCORES)

    wt_d = nc.dram_tensor("wt", [JC * I, ND], bf16, kind="ExternalInput")
    xblk_d = nc.dram_tensor("xblk", [64, NG * 128], bf16, kind="ExternalInput")
    xbl8_d = nc.dram_tensor("xbl8", [128, CH8 * B], bf16, kind="ExternalInput")
    ones_d = nc.dram_tensor("ones4", [128, B], bf16, kind="ExternalInput")
    v_d = nc.dram_tensor("v", [B, ND], f32, kind="ExternalOutput")

    with tile.TileContext(nc) as tc:
        with (
            tc.tile_pool(name="persist", bufs=1) as pp,
            tc.tile_pool(name="wstream", bufs=4) as wp,
            tc.tile_pool(name="work", bufs=2) as wk,
            tc.tile_pool(name="t2p", bufs=2) as t2p,
            tc.tile_pool(name="small", bufs=4) as sm,
            tc.tile_pool(name="zp", bufs=2) as zp,
            tc.tile_pool(name="ups", bufs=3, space="PSUM") as ups_pool,
            tc.tile_pool(name="sps", bufs=1, space="PSUM") as sps_pool,
            tc.tile_pool(name="dram", bufs=1, space="DRAM") as dr,
        ):
            xblk = pp.tile([64, NG, 128], bf16)
            nc.sync.dma_start(xblk[:], xblk_d[:])
            xbl8 = pp.tile([128, CH8, B], bf16)
            nc.sync.dma_start(xbl8[:], xbl8_d[:])
            ones4 = pp.tile([128, B], bf16)
            nc.sync.dma_start(ones4[:], ones_d[:])

            # logits [128=(j4,b), g, h, n'] bf16 (persistent across sweeps)
            logits = pp.tile([128, NG, 2, NH], bf16)
            v_rep = pp.tile([128, 2048], bf16)    # v in (h,d,n'') layout
            v_small = pp.tile([B, 2048], bf16)
            s_sb = pp.tile([128, 512], f32)       # strips (q=(h,nh), b)
            v_sb = pp.tile([B, ND], f32)

            cc_in = dr.tile([128, 512], f32)
            cc_out = dr.tile([128, 512], f32)

            def squash_update(r, s_ps):
                """AllReduce s strips, squash, write v_rep (or final v).

                Strip layout: partition 32*q + b, q = 2*h + nh;
                free = (d32, n16) d-major; n = h*32 + nh*16 + n'.
                s_ps None => cc_in already written (r0 path).
                """
                if s_ps is not None:
                    s_evac = sm.tile([128, 512], f32, name="s_evac", tag="sev")
                    nc.vector.tensor_copy(s_evac[:], s_ps[:])
                    nc.sync.dma_start(cc_in[:], s_evac[:])
                nc.gpsimd.collective_compute(
                    "AllReduce", ADD,
                    replica_groups=[list(range(NCORES))],
                    ins=[cc_in[:].opt()], outs=[cc_out[:].opt()],
                )
                nc.sync.dma_start(s_sb[:], cc_out[:])

                s3 = s_sb[:].rearrange("p (d n) -> p d n", d=D)  # [128,32,16]
                sq = sm.tile([128, D, 16], f32, name="sq", tag="sq")
                nc.vector.tensor_tensor(sq[:], s3, s3, op=MUL)
                nc.vector.tensor_tensor(
                    sq[:, 0:16, :], sq[:, 0:16, :], sq[:, 16:32, :], op=ADD)
                nc.vector.tensor_tensor(
                    sq[:, 0:8, :], sq[:, 0:8, :], sq[:, 8:16, :], op=ADD)
                nc.vector.tensor_tensor(
                    sq[:, 0:4, :], sq[:, 0:4, :], sq[:, 4:8, :], op=ADD)
                nc.vector.tensor_tensor(
                    sq[:, 0:2, :], sq[:, 0:2, :], sq[:, 2:4, :], op=ADD)
                ns2 = sm.tile([128, 16], f32, name="ns2", tag="ns2")
                nc.vector.tensor_tensor(ns2[:], sq[:, 0, :], sq[:, 1, :], op=ADD)

                onep = sm.tile([128, 16], f32, name="onep", tag="onep")
                nc.vector.tensor_scalar_add(onep[:], ns2[:], 1.0)
                eps_t = sm.tile([128, 1], f32, name="eps", tag="eps")
                nc.vector.memset(eps_t[:], EPS)
                rt = sm.tile([128, 16], f32, name="rt", tag="rt")
                nc.scalar.activation(rt[:], ns2[:], AF.Sqrt, bias=eps_t[:])
                den = sm.tile([128, 16], f32, name="den", tag="den")
                nc.vector.tensor_tensor(den[:], onep[:], rt[:], op=MUL)
                dinv = sm.tile([128, 16], f32, name="dinv", tag="dinv")
                nc.vector.reciprocal(dinv[:], den[:])
                scl = sm.tile([128, 16], f32, name="scl", tag="scl")
                nc.vector.tensor_tensor(scl[:], ns2[:], dinv[:], op=MUL)
                v4 = sm.tile([128, D, 16], f32, name="v4", tag="v4")
                nc.vector.tensor_tensor(
                    v4[:], s3,
                    scl[:, None, :].broadcast_to([128, D, 16]),
                    op=MUL)

                if r < NROUT - 1:
                    v4b = sm.tile([128, 512], bf16, name="v4b", tag="v4b")
                    nc.vector.tensor_copy(
                        v4b[:], v4[:].rearrange("p a b -> p (a b)"))
                    # strips -> v_small[b, (h, d, nh, n')] = (h, d, n'')
                    vsm = v_small[:].rearrange(
                        "b (h d nh n) -> b h d nh n", h=2, d=D, nh=2)
                    for h in range(2):
                        for nh in range(2):
                            q = 2 * h + nh
                            nc.sync.dma_start(
                                vsm[:, h, :, nh, :],
                                v4b[32 * q:32 * q + 32, :]
                                .rearrange("b (d n) -> b d n", d=D))
                    for rr in range(GRP):
                        nc.sync.dma_start(
                            v_rep[32 * rr:32 * rr + 32, :], v_small[:])
                else:
                    # final v in standard (n, d) layout
                    v4t = sm.tile([128, 16, D], f32, name="v4t", tag="v4t")
                    nc.vector.tensor_copy(
                        v4t[:], v4[:].rearrange("p d n -> p n d"))
                    vo = v_sb[:].rearrange(
                        "b (h nh n d) -> b h nh n d", h=2, nh=2, n=16)
                    for h in range(2):
                        for nh in range(2):
                            q = 2 * h + nh
                            nc.sync.dma_start(
                                vo[:, h, nh, :, :],
                                v4t[32 * q:32 * q + 32, :, :])

            for r in range(NROUT):
                if r == 0:
                    # accumulate s0*N = sum_j u_hat in PSUM partitions 0:32
                    acc = [ups_pool.tile([128, 1024], f32,
                                         name=f"acc{h}", tag="u_ps")
                           for h in range(2)]
                    for c in range(CH8):
                        w_t = wp.tile([128, 2048], bf16, name="w8", tag="w8")
                        nc.sync.dma_start(
                            w_t[:], wt_d[128 * c:128 * (c + 1), :])
                        for h in range(2):
                            for cch in range(2):
                                lo = 1024 * h + 512 * cch
                                nc.tensor.matmul(
                                    acc[h][0:32, 512 * cch:512 * (cch + 1)],
                                    xbl8[:, c, :],
                                    w_t[:, lo:lo + 512],
                                    start=(c == 0), stop=(c == CH8 - 1),
                                )
                    a_ev = wk.tile([32, 2, 1024], f32, name="a_ev", tag="a_ev")
                    for h in range(2):
                        nc.vector.tensor_scalar_mul(
                            a_ev[:, h, :], acc[h][0:32, :], 1.0 / N)
                    aev = a_ev[:].rearrange(
                        "b h (d nh n) -> b h d nh n", d=D, nh=2)
                    cci = cc_in[:].rearrange("(q b) (d n) -> q b d n", q=4, d=D)
                    for h in range(2):
                        for nh in range(2):
                            q = 2 * h + nh
                            nc.sync.dma_start(
                                cci[q, :, :, :], aev[:, h, :, nh, :])
                    squash_update(0, None)
                else:
                    s_ps = sps_pool.tile([128, 512], f32,
                                         name=f"s{r}", tag="s_ps")
                    pending_smm = []
                    for q4 in range(NG // QUAD):
                        u_sb4 = wk.tile([128, QUAD, 2048], bf16,
                                        name="u_sb4", tag="u_sb")
                        tl4 = wk.tile([128, 2 * QUAD, D, NH], bf16,
                                      name="tl4", tag="tl4")
                        for gi in range(QUAD):
                            g = q4 * QUAD + gi
                            w_t = wp.tile([64, 2048], bf16, name="w4", tag="w")
                            nc.sync.dma_start(
                                w_t[:], wt_d[64 * g:64 * (g + 1), :])
                            for h in range(2):
                                u_ps = ups_pool.tile([128, 1024], f32,
                                                     name="u_ps", tag="u_ps")
                                for cch in range(2):
                                    lo = 1024 * h + 512 * cch
                                    nc.tensor.matmul(
                                        u_ps[:, 512 * cch:512 * (cch + 1)],
                                        xblk[:, g, :],
                                        w_t[:, lo:lo + 512],
                                        start=True, stop=True,
                                    )
                                if pending_smm:
                                    pending_smm.pop(0)()
                                nc.scalar.activation(
                                    u_sb4[:, gi, 1024 * h:1024 * (h + 1)],
                                    u_ps[:], AF.Copy)
                            with nc.allow_low_precision("bf16 logits"):
                                nc.vector.tensor_tensor(
                                    tl4[:, 2 * gi:2 * gi + 2, :, :],
                                    u_sb4[:, gi, :].rearrange(
                                        "p (h d n) -> p h d n", h=2, d=D),
                                    v_rep[:].rearrange(
                                        "p (h d n) -> p h d n", h=2, d=D),
                                    op=MUL)

                        # fold over d: [128, 8, 32, 32] -> per-(g,h) rows
                        with nc.allow_low_precision("bf16 logits"):
                            nc.vector.tensor_tensor(
                                tl4[:, :, 0:16, :], tl4[:, :, 0:16, :],
                                tl4[:, :, 16:32, :], op=ADD)
                            nc.vector.tensor_tensor(
                                tl4[:, :, 0:8, :], tl4[:, :, 0:8, :],
                                tl4[:, :, 8:16, :], op=ADD)
                            nc.vector.tensor_tensor(
                                tl4[:, :, 0:4, :], tl4[:, :, 0:4, :],
                                tl4[:, :, 4:8, :], op=ADD)
                            nc.vector.tensor_tensor(
                                tl4[:, :, 0:2, :], tl4[:, :, 0:2, :],
                                tl4[:, :, 2:4, :], op=ADD)
                            for gi in range(QUAD):
                                g = q4 * QUAD + gi
                                dst = logits[:, g, :, :]
                                a0 = tl4[:, 2 * gi:2 * gi + 2, 0, :]
                                a1 = tl4[:, 2 * gi:2 * gi + 2, 1, :]
                                if r == 1:
                                    nc.vector.tensor_tensor(dst, a0, a1, op=ADD)
                                else:
                                    nc.vector.tensor_tensor(a0, a0, a1, op=ADD)
                                    nc.vector.tensor_tensor(dst, dst, a0, op=ADD)

                        for gi in range(QUAD):
                            g = q4 * QUAD + gi
                            e_t = sm.tile([128, 2, NH], f32,
                                          name="e_t", tag="e_t")
                            zsum = sm.tile([128, 1], f32, name="zs", tag="zs")
                            nc.scalar.activation(
                                e_t[:], logits[:, g, :, :], AF.Exp,
                                accum_out=zsum[:])
                            zrec = sm.tile([128, 1], f32, name="zr", tag="zr")
                            nc.vector.reciprocal(zrec[:], zsum[:])
                            c_t = zp.tile([128, 2, NH], bf16, name="ct", tag="ct")
                            nc.vector.tensor_scalar_mul(c_t[:], e_t[:], zrec[:])

                            # t2 = c (x) u_hat; d-outer bcast = DVE 2x.
                            # gpsimd is slower than DVE 2x: give it the h1
                            # half only on even groups.
                            t2 = t2p.tile([128, 2, D, NH], bf16,
                                          name="t2", tag="t2")
                            u3 = u_sb4[:, gi, :].rearrange(
                                "p (h d n) -> p h d n", h=2, d=D)
                            nc.vector.tensor_tensor(
                                t2[:, 0, :, :], u3[:, 0, :, :],
                                c_t[:, 0, None, :].broadcast_to([128, D, NH]),
                                op=MUL)
                            eng1 = nc.gpsimd if g % 2 == 0 else nc.vector
                            eng1.tensor_tensor(
                                t2[:, 1, :, :], u3[:, 1, :, :],
                                c_t[:, 1, None, :].broadcast_to([128, D, NH]),
                                op=MUL)

                            def make_smm(t2=t2, g=g):
                                def emit():
                                    for h in range(2):
                                        for nh in range(2):
                                            qq = 2 * h + nh
                                            nc.tensor.matmul(
                                                s_ps[32 * qq:32 * qq + 32, :],
                                                ones4[:],
                                                t2[:, h, :, 16 * nh:16 * nh + 16],
                                                start=(g == 0),
                                                stop=(g == NG - 1),
                                                tile_position=(0, 32 * qq),
                                                skip_group_check=True,
                                            )
                                return emit
                            pending_smm.append(make_smm())
                    while pending_smm:
                        pending_smm.pop(0)()
                    squash_update(r, s_ps)

            nc.sync.dma_start(v_d[:], v_sb[:])

    nc.compile()
    return nc


def _prep_inputs(x: np.ndarray, W: np.ndarray):
    import ml_dtypes
    bf = ml_dtypes.bfloat16
    in_maps = []
    ones4 = np.tile(np.eye(B, dtype=np.float32), (GRP, 1)).astype(bf)
    for k in range(NCORES):
        jlo, jhi = k * JC, (k + 1) * JC
        Wc = W[:, jlo:jhi]                      # [N, JC, D, I]
        # wt[(j,i), (h,d,n')] = W[h*32+n', j, d, i]
        wt = np.ascontiguousarray(
            Wc.reshape(2, NH, JC, D, I).transpose(2, 4, 0, 3, 1)
        ).reshape(JC * I, ND).astype(bf)
        xc = x[:, jlo:jhi, :]                   # [B, JC, I]
        xg = np.ascontiguousarray(
            xc.transpose(1, 2, 0)).reshape(NG, GRP, I, B)  # [g, rr, i, b]
        # xblk[g][(rr,i), (rr',b)] = x[b, 4g+rr, i] * delta(rr,rr')
        xblk = np.zeros((NG, GRP * I, GRP * B), dtype=np.float32)
        for rr in range(GRP):
            xblk[:, rr * I:(rr + 1) * I, rr * B:(rr + 1) * B] = xg[:, rr]
        xblk = np.ascontiguousarray(xblk.transpose(1, 0, 2)).reshape(
            GRP * I, NG * GRP * B).astype(bf)           # [64, NG*128]
        # xbl8[(jj,i), (c,b)] = x[b, 8c+jj, i] (dense)
        x8 = np.ascontiguousarray(
            xc.transpose(1, 2, 0)).reshape(CH8, 8, I, B)   # [c, jj, i, b]
        xbl8 = np.ascontiguousarray(
            x8.transpose(1, 2, 0, 3)).reshape(128, CH8 * B).astype(bf)
        in_maps.append({
            "wt": wt,
            "xblk": xblk,
            "xbl8": xbl8,
            "ones4": ones4,
        })
    return in_maps


def kernel(x: np.ndarray, W: np.ndarray) -> np.ndarray:
    from concourse.bass_utils import run_bass_kernel_spmd

    nc = _build()
    in_maps = _prep_inputs(x, W)
    res = run_bass_kernel_spmd(nc, in_maps, list(range(NCORES)))
    v = np.asarray(res.results[0]["v"], dtype=np.float32)
    return v.reshape(B, N, D)


if __name__ == "__main__":
    rng = np.random.default_rng(0)
    x = rng.normal(size=(B, J, I)).astype(np.float32)
    W = rng.normal(size=(N, J, D, I)).astype(np.float32) * 0.05
    v = kernel(x, W)
    print(v.shape, v.dtype, np.abs(v).max())
